# revision 2
# baseline (speedup 1.0000x reference)
"""Distributed causal self-attention for 8 Trainium2 NeuronCores.

Problem: x[2,2048,1024] @ w_qkv[1024,3072] -> causal MHA (16 heads, d=64)
         -> @ w_out[1024,1024]. All fp32.

Sharding: core c (0..7) handles batch b=c//4 and head group g=c%4 (4 heads).
Each core projects qkv for its heads, runs flash attention (transposed-score
layout), then an AllToAll within each 4-core batch group converts head-
parallel attention output into token-parallel slices for the output
projection.  Core c writes output rows [b, 512*g : 512*(g+1), :].

Matmuls run in float32r (TF32-like, full PE rate); softmax in fp32.
"""

import sys

for _p in ("/opt/trn_rl_repo", "/root/.axon_site/_ro/trn_rl_repo"):
    if _p not in sys.path:
        sys.path.insert(0, _p)

import numpy as np

import concourse.bass as bass  # noqa: F401  (bass types used via tile/bacc)
import concourse.mybir as mybir
import concourse.tile as tile
from concourse import bacc
from concourse.bass_utils import run_bass_kernel_spmd

P = 128
B, T, C = 2, 2048, 1024
H, D = 16, 64
HL = 4               # heads per core
DL = HL * D          # 256 local head dims
KC = C // P          # 8 contraction tiles over C
QB = 512             # query chunk
NQ = T // QB         # 4 query chunks
NT = T // P          # 16 token tiles
G = 4                # cores per batch group
TS = T // G          # 512-token output slice per core
SCALE = 1.0 / 8.0    # 1/sqrt(64)
NEG = -1.0e30

F32 = mybir.dt.float32
F32R = mybir.dt.float32r

_CACHED = {}


def _mask_data():
    # tril mask: 0 where key j <= query i, NEG above the diagonal
    j = np.arange(P)[:, None]
    i = np.arange(P)[None, :]
    return np.where(j <= i, 0.0, NEG).astype(np.float32)


def _build():
    nc = bacc.Bacc("TRN2", target_bir_lowering=False, debug=False,
                   num_devices=8)

    xT = nc.dram_tensor("xT", [C, T], F32R, kind="ExternalInput")
    wq = nc.dram_tensor("wq", [C, DL], F32R, kind="ExternalInput")
    wk = nc.dram_tensor("wk", [C, DL], F32R, kind="ExternalInput")
    wv = nc.dram_tensor("wv", [C, DL], F32R, kind="ExternalInput")
    bq = nc.dram_tensor("bq", [1, DL], F32R, kind="ExternalInput")
    bk = nc.dram_tensor("bk", [1, DL], F32R, kind="ExternalInput")
    bv = nc.dram_tensor("bv", [1, DL], F32R, kind="ExternalInput")
    wo = nc.dram_tensor("wo", [DL, C], F32R, kind="ExternalInput")
    bo = nc.dram_tensor("bo", [1, C], F32R, kind="ExternalInput")
    # per query-chunk ReduceScatter slices: rows qc*512 + g*128 .. +128
    out = nc.dram_tensor("out", [NQ, P, C], F32, kind="ExternalOutput")

    masks_dram = nc.inline_tensor(_mask_data(), name="cmasks")

    with tile.TileContext(nc) as tc:
        with (
            tc.tile_pool(name="const", bufs=1) as cp,
            tc.tile_pool(name="persist", bufs=1) as pp,
            tc.tile_pool(name="work", bufs=3) as wk_p,
            tc.tile_pool(name="dram", bufs=1, space="DRAM") as dp,
            tc.tile_pool(name="ps_proj", bufs=2, space="PSUM") as ps_proj,
            tc.tile_pool(name="ps_sT", bufs=2, space="PSUM") as ps_sT,
            tc.tile_pool(name="ps_pv", bufs=2, space="PSUM") as ps_pv,
        ):
            # ---- constants ----
            masks = cp.tile([P, P], F32)
            nc.sync.dma_start(masks[:], masks_dram[:])
            ones_f = cp.tile([1, QB], F32)
            nc.vector.memset(ones_f[:], 1.0)
            ones_r = cp.tile([1, QB], F32R)
            nc.vector.tensor_copy(ones_r[:], ones_f[:])
            # q/k biases as per-partition columns [128, 2] (mi-major)
            bq_col = cp.tile([P, 2], F32)
            bk_col = cp.tile([P, 2], F32)
            nc.sync.dma_start(
                bq_col[:], bq.bitcast(F32)[0, :].rearrange("(m p) -> p m", p=P))
            nc.sync.dma_start(
                bk_col[:], bk.bitcast(F32)[0, :].rearrange("(m p) -> p m", p=P))
            bv_sb = cp.tile([1, DL], F32R)
            bo_sb = cp.tile([1, C], F32R)
            nc.sync.dma_start(bv_sb[:], bv[:])
            nc.sync.dma_start(bo_sb[:], bo[:])

            # ---- persistent activations ----
            qT_sb = pp.tile([P, 2, T], F32R)     # [d, t], d = mi*128+p
            kT_sb = pp.tile([P, 2, T], F32R)
            v_sb = pp.tile([P, NT, HL * (D + 1)], F32R)  # per head: 64 v + ones
            aoT_sb = pp.tile([P, 2, T], F32R)    # attention out^T (normalized)

            # ones columns of v_sb (softmax denominator accumulator)
            ones64 = cp.tile([P, NT * HL], F32)
            nc.vector.memset(ones64[:], 1.0)
            vones = v_sb.rearrange("p n (h e) -> p n h e", h=HL)[:, :, :, D:D + 1]
            nc.vector.tensor_copy(vones, ones64[:].rearrange(
                "p (n h) -> p n h", n=NT)[:, :, :, None])

            with tc.tile_pool(name="xw", bufs=1) as xw:
                xTr = xw.tile([P, KC, T], F32R)
                for kk in range(KC):
                    nc.sync.dma_start(
                        xTr[:, kk, :],
                        xT.rearrange("(k p) t -> k p t", p=P)[kk])
                wq_sb = xw.tile([P, KC, DL], F32R)
                wk_sb = xw.tile([P, KC, DL], F32R)
                wv_sb = xw.tile([P, KC, DL], F32R)
                nc.sync.dma_start(wq_sb[:], wq.rearrange("(k p) m -> p k m", p=P))
                nc.sync.dma_start(wk_sb[:], wk.rearrange("(k p) m -> p k m", p=P))
                nc.sync.dma_start(wv_sb[:], wv.rearrange("(k p) m -> p k m", p=P))

                # ---- phase A: qkv projection ----
                for w_sb, b_col, dst in ((wq_sb, bq_col, qT_sb),
                                         (wk_sb, bk_col, kT_sb)):
                    for mi in range(2):
                        for ni in range(NQ):
                            ps = ps_proj.tile([P, QB], F32, name="proj_ps",
                                              tag="proj_ps")
                            for kk in range(KC):
                                nc.tensor.matmul(
                                    ps[:],
                                    w_sb[:, kk, mi * P:(mi + 1) * P],
                                    xTr[:, kk, ni * QB:(ni + 1) * QB],
                                    start=(kk == 0), stop=(kk == KC - 1))
                            nc.vector.tensor_scalar_add(
                                dst[:, mi, ni * QB:(ni + 1) * QB], ps[:],
                                b_col[:, mi:mi + 1])
                for ti in range(NT):
                    ps = ps_proj.tile([P, DL], F32, name="proj_ps",
                                      tag="proj_ps")
                    for kk in range(KC):
                        nc.tensor.matmul(ps[:], xTr[:, kk, ti * P:(ti + 1) * P],
                                         wv_sb[:, kk, :],
                                         start=(kk == 0), stop=False)
                    nc.tensor.matmul(ps[:], ones_r[:, :P], bv_sb[:],
                                     start=False, stop=True)
                    nc.vector.tensor_copy(
                        v_sb.rearrange("p n (h e) -> p n h e", h=HL)
                        [:, ti, :, 0:D],
                        ps[:].rearrange("p (h e) -> p h e", e=D))
            # xw pool released; wo loads overlap attention below.

            with tc.tile_pool(name="wo_pool", bufs=1) as wop:
                wo_sb = wop.tile([P, 2, C], F32R)
                nc.sync.dma_start(wo_sb[:],
                                  wo.rearrange("(k p) n -> p k n", p=P))
                bo_bc = wop.tile([P, C], F32)
                nc.gpsimd.partition_broadcast(bo_bc[:], bo_sb[:].bitcast(F32))

                # ---- phases B+C interleaved per query chunk ----
                BF16 = mybir.dt.bfloat16
                part_dram = dp.tile([T, C], BF16)
                rs_out = dp.tile([NQ, P, C], BF16)

                def outproj_jobs(qc):
                    # 8 projection psum-groups + deferred RS for chunk qc;
                    # emitted one at a time inside the NEXT chunk's attention
                    # stream as exp-independent PE gap filler.
                    jobs = []

                    def group(mi2, ni):
                        ps = ps_proj.tile([P, QB], F32, name="proj_ps",
                                          tag="proj_ps")
                        for kk in range(2):
                            nc.tensor.matmul(
                                ps[:],
                                aoT_sb[:, kk, mi2 * P:(mi2 + 1) * P],
                                wo_sb[:, kk, ni * QB:(ni + 1) * QB],
                                start=(kk == 0), stop=(kk == 1))
                        o_sb = wk_p.tile([P, QB], BF16, name="o_sb",
                                         tag="o_sb", bufs=2)
                        nc.scalar.copy(o_sb[:], ps[:])
                        nc.sync.dma_start(
                            part_dram[mi2 * P:(mi2 + 1) * P,
                                      ni * QB:(ni + 1) * QB],
                            o_sb[:])

                    for mi2 in range(4 * qc, 4 * qc + 4):
                        for ni in range(2):
                            jobs.append(lambda mi2=mi2, ni=ni:
                                        group(mi2, ni))

                    def rs_job():
                        nc.gpsimd.collective_compute(
                            "ReduceScatter",
                            mybir.AluOpType.add,
                            replica_groups=[[0, 1, 2, 3], [4, 5, 6, 7]],
                            ins=[part_dram[qc * QB:(qc + 1) * QB, :]],
                            outs=[rs_out[qc]],
                        )
                        r_sb = wk_p.tile([P, C], BF16, name="r_sb",
                                         tag="r_sb", bufs=2)
                        nc.sync.dma_start(r_sb[:], rs_out[qc])
                        f_sb = wk_p.tile([P, C], F32, name="f_sb",
                                         tag="f_sb", bufs=2)
                        nc.vector.tensor_add(f_sb[:], r_sb[:], bo_bc[:])
                        nc.sync.dma_start(out[qc], f_sb[:])

                    jobs.append(rs_job)
                    return jobs

                pending = []

                def drain():
                    if pending:
                        pending.pop(0)()

                for qc in range(NQ):
                    # flash attention: head pairs interleaved at the
                    # key-block level so PE stays busy while ACT runs exp
                    nkb = 4 * qc + 4
                    for hp in range(HL // 2):
                        heads = (2 * hp, 2 * hp + 1)
                        pvs = {}
                        for h in heads:
                            pvs[h] = ps_pv.tile([P, QB], F32,
                                                name="pv_ps", tag="pv")
                        # full (unmasked) key-block pairs
                        for kp in range(2 * qc):
                            for h in heads:
                                po = 64 * (h % 2)
                                mi = h // 2
                                sT = ps_sT.tile([P, 2 * QB], F32,
                                                name="sT_ps", tag="sT")
                                for half in range(2):
                                    kb = 2 * kp + half
                                    nc.tensor.matmul(
                                        sT[:, half * QB:(half + 1) * QB],
                                        kT_sb[po:po + D, mi,
                                              kb * P:(kb + 1) * P],
                                        qT_sb[po:po + D, mi,
                                              qc * QB:(qc + 1) * QB],
                                        start=True, stop=True)
                                pT = wk_p.tile([P, 2 * QB], F32R, name="pT",
                                               tag="pT")
                                nc.scalar.activation(
                                    pT[:], sT[:],
                                    mybir.ActivationFunctionType.Exp,
                                    scale=SCALE)
                                for half in range(2):
                                    kb = 2 * kp + half
                                    nc.tensor.matmul(
                                        pvs[h][0:D + 1, :],
                                        v_sb[:, kb,
                                             h * (D + 1):(h + 1) * (D + 1)],
                                        pT[:, half * QB:(half + 1) * QB],
                                        start=(kb == 0), stop=False)
                                drain()
                        # diagonal blocks, queries narrowed to the visible
                        # range [128*di, 512); only a [128,128] tril masked
                        for di in range(4):
                            kb = 4 * qc + di
                            q0 = di * P          # first visible query col
                            qw = QB - q0
                            for h in heads:
                                po = 64 * (h % 2)
                                mi = h // 2
                                sT = ps_sT.tile([P, 2 * QB], F32,
                                                name="sT_ps", tag="sT")
                                nc.tensor.matmul(
                                    sT[:, 0:qw],
                                    kT_sb[po:po + D, mi,
                                          kb * P:(kb + 1) * P],
                                    qT_sb[po:po + D, mi,
                                          qc * QB + q0:(qc + 1) * QB],
                                    start=True, stop=True)
                                nc.vector.tensor_add(
                                    sT[:, 0:P], sT[:, 0:P], masks[:])
                                pT = wk_p.tile([P, 2 * QB], F32R, name="pT",
                                               tag="pT")
                                nc.scalar.activation(
                                    pT[:, 0:qw], sT[:, 0:qw],
                                    mybir.ActivationFunctionType.Exp,
                                    scale=SCALE)
                                nc.tensor.matmul(
                                    pvs[h][0:D + 1, q0:QB],
                                    v_sb[:, kb,
                                         h * (D + 1):(h + 1) * (D + 1)],
                                    pT[:, 0:qw],
                                    start=(qc == 0 and di == 0),
                                    stop=(di == 3))
                                drain()
                        for h in heads:
                            po = 64 * (h % 2)
                            mi = h // 2
                            rbc = wk_p.tile([D, QB], F32, name="rbc",
                                            tag="rbc", bufs=2)
                            lrow = wk_p.tile([1, QB], F32, name="lrow",
                                             tag="lrow", bufs=2)
                            nc.scalar.copy(lrow[:], pvs[h][D:D + 1, :])
                            nc.vector.reciprocal_approx_fast(
                                out=rbc[0:1, :], in_=lrow[:])
                            nc.gpsimd.partition_broadcast(rbc[:], rbc[0:1, :])
                            nc.vector.tensor_mul(
                                aoT_sb[po:po + D, mi, qc * QB:(qc + 1) * QB],
                                pvs[h][0:D, :], rbc[:])

                    # queue this chunk's output projection + RS; they are
                    # emitted inside the next chunk's attention stream
                    assert not pending
                    pending = outproj_jobs(qc)

                # drain the last chunk's jobs
                while pending:
                    drain()

    nc.compile()
    return nc


def kernel(x, w_qkv, b_qkv, w_out, b_out):
    x = np.ascontiguousarray(np.asarray(x, dtype=np.float32))
    w_qkv = np.asarray(w_qkv, dtype=np.float32)
    b_qkv = np.asarray(b_qkv, dtype=np.float32)
    w_out = np.ascontiguousarray(np.asarray(w_out, dtype=np.float32))
    b_out = np.asarray(b_out, dtype=np.float32)

    if "nc" not in _CACHED:
        _CACHED["nc"] = _build()
    nc = _CACHED["nc"]

    xTs = [np.ascontiguousarray(x[b_].T) for b_ in range(B)]
    bo = np.ascontiguousarray(b_out[None, :])
    in_maps = []
    for c in range(8):
        b_, g = c // 4, c % 4
        sl = slice(g * DL, (g + 1) * DL)
        in_maps.append({
            "xT": xTs[b_],
            "wq": np.ascontiguousarray(w_qkv[:, 0 * C:1 * C][:, sl]),
            "wk": np.ascontiguousarray(w_qkv[:, 1 * C:2 * C][:, sl]),
            "wv": np.ascontiguousarray(w_qkv[:, 2 * C:3 * C][:, sl]),
            "bq": np.ascontiguousarray(b_qkv[0 * C:1 * C][sl][None, :]),
            "bk": np.ascontiguousarray(b_qkv[1 * C:2 * C][sl][None, :]),
            "bv": np.ascontiguousarray(b_qkv[2 * C:3 * C][sl][None, :]),
            "wo": np.ascontiguousarray(w_out[g * DL:(g + 1) * DL, :]),
            "bo": bo,
        })
    res = run_bass_kernel_spmd(nc, in_maps, list(range(8)))
    _CACHED["last_result"] = res
    out_full = np.empty((B, T, C), dtype=np.float32)
    for c in range(8):
        b_, g = c // 4, c % 4
        o = res.results[c]["out"]          # [NQ, P, C]
        for qc in range(NQ):
            r0 = qc * QB + g * P
            out_full[b_, r0:r0 + P, :] = o[qc]
    return out_full



# revision 10
# speedup vs baseline: 1.1206x; 1.1206x over previous
"""Distributed causal self-attention for 8 Trainium2 NeuronCores.

Problem: x[2,2048,1024] @ w_qkv[1024,3072] -> causal MHA (16 heads, d=64)
         -> @ w_out[1024,1024]. All fp32 in/out.

Sharding: core c (0..7) handles batch b=c//4 and head group g=c%4 (4 heads).
Each core projects qkv for its heads, runs flash attention (transposed-score
layout), then a ReduceScatter within each 4-core batch group combines the
partial output projections.  Core c owns output rows [b, qc*512+g*128, :].

v2 layout: all matmul operands in bf16 (fp32 PSUM accumulation, fp32
softmax math).  Projection is streamed per 512-token chunk so attention
for chunk qc overlaps the DMA + projection of chunk qc+1.  Score matmuls
for a head pair run row-packed (tile rows 0-63 / 64-127 concurrently);
pv matmuls are split per 64-key half and cross-packed over the pair.
"""

import sys

for _p in ("/opt/trn_rl_repo", "/root/.axon_site/_ro/trn_rl_repo"):
    if _p not in sys.path:
        sys.path.insert(0, _p)

import ml_dtypes
import numpy as np

import concourse.bass as bass  # noqa: F401
import concourse.mybir as mybir
import concourse.tile as tile
from concourse import bacc
from concourse.bass_utils import run_bass_kernel_spmd

P = 128
B, T, C = 2, 2048, 1024
H, D = 16, 64
HL = 4               # heads per core
DL = HL * D          # 256 local head dims
KC = C // P          # 8 contraction tiles over C
QB = 512             # query chunk
NQ = T // QB         # 4 query chunks
NT = T // P          # 16 token tiles
G = 4                # cores per batch group
SCALE = 1.0 / 8.0    # 1/sqrt(64)
NEG = -1.0e30
N_WARM = 10          # PE warm-up dummy matmuls

F32 = mybir.dt.float32
BF16 = mybir.dt.bfloat16
BF_NP = ml_dtypes.bfloat16

import os as _os
# conservative-mode bisection flags (1 = use baseline-style construct)
C_RECIP = _os.environ.get("K_C_RECIP", "0") == "1"   # ACT copy before recip
C_MASK = _os.environ.get("K_C_MASK", "0") == "1"     # per-head mask adds
C_EXP = _os.environ.get("K_C_EXP", "0") == "1"       # per-head 2D exp
C_PV = _os.environ.get("K_C_PV", "1") == "1"         # unpacked full-row pv
# (cross-packed pv — two row-tiled matmuls accumulating into the same
# PSUM tile — crashes the device; keep full-row pv.)
C_WARM = _os.environ.get("K_C_WARM", "0") == "1"     # no warmup dummies

_CACHED = {}


def _mask_data():
    # tril mask: 0 where key j <= query i, NEG above the diagonal
    j = np.arange(P)[:, None]
    i = np.arange(P)[None, :]
    return np.where(j <= i, 0.0, NEG).astype(np.float32)


def _build():
    nc = bacc.Bacc("TRN2", target_bir_lowering=False, debug=False,
                   num_devices=8)

    xT = nc.dram_tensor("xT", [KC, P, T], BF16, kind="ExternalInput")
    wq = nc.dram_tensor("wq", [P, KC, DL], BF16, kind="ExternalInput")
    wk = nc.dram_tensor("wk", [P, KC, DL], BF16, kind="ExternalInput")
    wv = nc.dram_tensor("wv", [P, KC, DL], BF16, kind="ExternalInput")
    bq = nc.dram_tensor("bq", [P, 2], F32, kind="ExternalInput")
    bk = nc.dram_tensor("bk", [P, 2], F32, kind="ExternalInput")
    bv = nc.dram_tensor("bv", [1, DL], F32, kind="ExternalInput")
    wo = nc.dram_tensor("wo", [P, 2, C], BF16, kind="ExternalInput")
    bo = nc.dram_tensor("bo", [1, C], F32, kind="ExternalInput")
    # per query-chunk ReduceScatter slices: rows qc*512 + g*128 .. +128
    out = nc.dram_tensor("out", [NQ, P, C], F32, kind="ExternalOutput")

    masks_dram = nc.inline_tensor(_mask_data(), name="cmasks")

    with tile.TileContext(nc) as tc:
        with (
            tc.tile_pool(name="const", bufs=1) as cp,
            tc.tile_pool(name="persist", bufs=1) as pp,
            tc.tile_pool(name="work", bufs=3) as wk_p,
            tc.tile_pool(name="xchunk", bufs=2) as xp,
            tc.tile_pool(name="dram", bufs=1, space="DRAM") as dp,
            tc.tile_pool(name="ps_a", bufs=2, space="PSUM") as ps_a,
            tc.tile_pool(name="ps_sT", bufs=2, space="PSUM") as ps_sT,
            tc.tile_pool(name="ps_pv", bufs=2, space="PSUM") as ps_pv,
        ):
            # ---- small constants ----
            masks = cp.tile([P, P], F32)
            nc.sync.dma_start(masks[:], masks_dram[:])
            bq_col = cp.tile([P, 2], F32)
            bk_col = cp.tile([P, 2], F32)
            nc.sync.dma_start(bq_col[:], bq[:])
            nc.sync.dma_start(bk_col[:], bk[:])
            bv_sb = cp.tile([1, DL], F32)
            bo_sb = cp.tile([1, C], F32)
            nc.sync.dma_start(bv_sb[:], bv[:])
            nc.sync.dma_start(bo_sb[:], bo[:])
            bv_bc = cp.tile([P, DL], F32)
            nc.gpsimd.partition_broadcast(bv_bc[:], bv_sb[:])
            bo_bc = cp.tile([P, C], F32)
            nc.gpsimd.partition_broadcast(bo_bc[:], bo_sb[:])
            spin = cp.tile([P, QB], BF16)
            nc.vector.memset(spin[:], 0.0)

            # ---- persistent activations (all bf16) ----
            qT_sb = pp.tile([P, 2, T], BF16)     # [d, t], d = mi*128+p
            kT_sb = pp.tile([P, 2, T], BF16)
            v_sb = pp.tile([P, NT, HL * (D + 1)], BF16)  # per head: 64 v + 1
            aoT_sb = pp.tile([P, 2, T], BF16)    # attention out^T (normalized)

            # ones columns of v_sb (softmax denominator accumulator)
            ones64 = cp.tile([P, NT * HL], F32)
            nc.vector.memset(ones64[:], 1.0)
            vones = v_sb.rearrange("p n (h e) -> p n h e", h=HL)[:, :, :, D:D + 1]
            nc.vector.tensor_copy(vones, ones64[:].rearrange(
                "p (n h) -> p n h", n=NT)[:, :, :, None])

            # ---- weights: per-kk DMA so the first matmuls start early ----
            wq_sb = pp.tile([P, KC, DL], BF16)
            wk_sb = pp.tile([P, KC, DL], BF16)
            wv_sb = pp.tile([P, KC, DL], BF16)
            for kk in range(KC):
                nc.sync.dma_start(wq_sb[:, kk, :], wq[:, kk, :])
                nc.sync.dma_start(wk_sb[:, kk, :], wk[:, kk, :])
                nc.sync.dma_start(wv_sb[:, kk, :], wv[:, kk, :])
            wo_sb = pp.tile([P, 2, C], BF16)
            nc.sync.dma_start(wo_sb[:], wo[:])

            # ---- PE warm-up: keep the HAM activity window busy during the
            # initial weight/x DMAs so real matmuls start at 2.4 GHz.
            if not C_WARM:
                for _ in range(N_WARM):
                    psd = ps_a.tile([P, QB], F32, name="psA", tag="psA")
                    nc.tensor.matmul(psd[:], spin[:, 0:P], spin[:],
                                     start=True, stop=True)

            def load_chunk(ni):
                xc = xp.tile([P, KC, QB], BF16, name="xc", tag="xc")
                for kk in range(KC):
                    nc.sync.dma_start(xc[:, kk, :],
                                      xT[kk, :, ni * QB:(ni + 1) * QB])
                return xc

            part_dram = dp.tile([T, C], BF16)
            rs_out = dp.tile([NQ, P, C], BF16)

            pending = []

            def drain():
                if pending:
                    pending.pop(0)()

            def outproj_jobs(qc):
                # 8 projection psum-groups + deferred RS for chunk qc;
                # emitted one at a time inside the NEXT chunk's stream as
                # PE gap filler.
                jobs = []

                def group(mi2, nj):
                    ps = ps_a.tile([P, QB], F32, name="psA", tag="psA")
                    for kk in range(2):
                        nc.tensor.matmul(
                            ps[:],
                            aoT_sb[:, kk, mi2 * P:(mi2 + 1) * P],
                            wo_sb[:, kk, nj * QB:(nj + 1) * QB],
                            start=(kk == 0), stop=(kk == 1))
                    o_sb = wk_p.tile([P, QB], BF16, name="o_sb",
                                     tag="o_sb", bufs=2)
                    nc.vector.tensor_copy(o_sb[:], ps[:])
                    nc.sync.dma_start(
                        part_dram[mi2 * P:(mi2 + 1) * P,
                                  nj * QB:(nj + 1) * QB],
                        o_sb[:])

                for mi2 in range(4 * qc, 4 * qc + 4):
                    for nj in range(2):
                        jobs.append(lambda mi2=mi2, nj=nj: group(mi2, nj))

                def rs_job():
                    nc.gpsimd.collective_compute(
                        "ReduceScatter",
                        mybir.AluOpType.add,
                        replica_groups=[[0, 1, 2, 3], [4, 5, 6, 7]],
                        ins=[part_dram[qc * QB:(qc + 1) * QB, :]],
                        outs=[rs_out[qc]],
                    )
                    r_sb = wk_p.tile([P, C], BF16, name="r_sb",
                                     tag="r_sb", bufs=2)
                    nc.sync.dma_start(r_sb[:], rs_out[qc])
                    f_sb = wk_p.tile([P, C], F32, name="f_sb",
                                     tag="f_sb", bufs=2)
                    nc.vector.tensor_add(f_sb[:], r_sb[:], bo_bc[:])
                    nc.sync.dma_start(out[qc], f_sb[:])

                jobs.append(rs_job)
                return jobs

            xcs = [load_chunk(0)]
            for ni in range(NQ):
                if ni + 1 < NQ:
                    xcs.append(load_chunk(ni + 1))
                xc = xcs[ni]

                # ---- q/k projection for this token chunk ----
                for w_sb, b_col, dst in ((wq_sb, bq_col, qT_sb),
                                         (wk_sb, bk_col, kT_sb)):
                    for mi in range(2):
                        ps = ps_a.tile([P, QB], F32, name="psA", tag="psA")
                        for kk in range(KC):
                            nc.tensor.matmul(
                                ps[:],
                                w_sb[:, kk, mi * P:(mi + 1) * P],
                                xc[:, kk, :],
                                start=(kk == 0), stop=(kk == KC - 1))
                        nc.vector.tensor_scalar_add(
                            dst[:, mi, ni * QB:(ni + 1) * QB], ps[:],
                            b_col[:, mi:mi + 1])
                        drain()
                # ---- v projection (tokens stationary) ----
                for tj in range(4):
                    ti = 4 * ni + tj
                    ps = ps_a.tile([P, QB], F32, name="psA", tag="psA")
                    for kk in range(KC):
                        nc.tensor.matmul(
                            ps[:, 0:DL],
                            xc[:, kk, tj * P:(tj + 1) * P],
                            wv_sb[:, kk, :],
                            start=(kk == 0), stop=(kk == KC - 1))
                    nc.vector.tensor_add(
                        v_sb.rearrange("p n (h e) -> p n h e", h=HL)
                        [:, ti, :, 0:D],
                        ps[:, 0:DL].rearrange("p (h e) -> p h e", e=D),
                        bv_bc[:].rearrange("p (h e) -> p h e", e=D))
                    drain()

                # ---- attention for query chunk qc = ni ----
                qc = ni
                nkb = 4 * qc + 4
                for pair in range(2):
                    mi = pair
                    he, ho = 2 * pair, 2 * pair + 1
                    pv = [ps_pv.tile([P, QB], F32, name="pv", tag="pv")
                          for _ in range(2)]
                    for kb in range(nkb):
                        diag = kb >= 4 * qc
                        q0 = P * (kb - 4 * qc) if diag else 0
                        sT = ps_sT.tile([P, 2, QB], F32, name="sT", tag="sT")
                        for e in range(2):
                            po = 64 * e
                            nc.tensor.matmul(
                                sT[:, e, q0:QB],
                                kT_sb[po:po + D, mi, kb * P:(kb + 1) * P],
                                qT_sb[po:po + D, mi,
                                      qc * QB + q0:(qc + 1) * QB],
                                start=True, stop=True)
                        if diag:
                            if C_MASK:
                                for e in range(2):
                                    nc.vector.tensor_add(
                                        sT[:, e, q0:q0 + P],
                                        sT[:, e, q0:q0 + P], masks[:])
                            else:
                                nc.vector.tensor_add(
                                    sT[:, :, q0:q0 + P], sT[:, :, q0:q0 + P],
                                    masks[:][:, None, :].broadcast_to(
                                        (P, 2, P)))
                        pT = wk_p.tile([P, 2, QB], BF16, name="pT", tag="pT")
                        if C_EXP:
                            for e in range(2):
                                nc.scalar.activation(
                                    pT[:, e, q0:QB], sT[:, e, q0:QB],
                                    mybir.ActivationFunctionType.Exp,
                                    scale=SCALE)
                        else:
                            nc.scalar.activation(
                                pT[:, :, q0:QB], sT[:, :, q0:QB],
                                mybir.ActivationFunctionType.Exp, scale=SCALE)
                        first, last = (kb == 0), (kb == nkb - 1)
                        if C_PV:
                            for e, h in ((0, he), (1, ho)):
                                nc.tensor.matmul(
                                    pv[e][0:D + 1, q0:QB],
                                    v_sb[:, kb, h * (D + 1):(h + 1) * (D + 1)],
                                    pT[:, e, q0:QB],
                                    start=first, stop=last)
                        else:
                            # cross-packed pv: slot 0 = (he, keys 0:64) ||
                            # (ho, keys 64:128); slot 1 swaps the key halves.
                            for slot in range(2):
                                ke, ko = 64 * slot, 64 - 64 * slot
                                nc.tensor.matmul(
                                    pv[0][0:D + 1, q0:QB],
                                    v_sb[ke:ke + 64, kb,
                                         he * (D + 1):(he + 1) * (D + 1)],
                                    pT[ke:ke + 64, 0, q0:QB],
                                    start=(first and slot == 0),
                                    stop=(last and slot == 1))
                                nc.tensor.matmul(
                                    pv[1][0:D + 1, q0:QB],
                                    v_sb[ko:ko + 64, kb,
                                         ho * (D + 1):(ho + 1) * (D + 1)],
                                    pT[ko:ko + 64, 1, q0:QB],
                                    start=(first and slot == 0),
                                    stop=(last and slot == 1))
                        drain()
                    # ---- normalize the pair: 1/l broadcast, ao = pv * r ----
                    rr = wk_p.tile([1, 2, QB], F32, name="rr", tag="rr",
                                   bufs=2)
                    # reciprocal_approx_fast must NOT read PSUM directly
                    # (silently wrong on HW); stage the row through SBUF.
                    lrow = wk_p.tile([1, 2, QB], F32, name="lrow",
                                     tag="lrow", bufs=2)
                    for e in range(2):
                        nc.vector.tensor_copy(lrow[0:1, e, :],
                                              pv[e][D:D + 1, :])
                        nc.vector.reciprocal_approx_fast(
                            out=rr[0:1, e, :], in_=lrow[0:1, e, :])
                    rbc = wk_p.tile([P, 2, QB], F32, name="rbc", tag="rbc",
                                    bufs=2)
                    nc.gpsimd.partition_broadcast(rbc[:], rr[:])
                    for e in range(2):
                        po = 64 * e
                        nc.vector.tensor_mul(
                            aoT_sb[po:po + D, mi, qc * QB:(qc + 1) * QB],
                            pv[e][0:D, :], rbc[po:po + D, e, :])

                assert not pending
                pending = outproj_jobs(qc)

            # drain the last chunk's jobs
            while pending:
                drain()

    nc.compile()
    return nc


def kernel(x, w_qkv, b_qkv, w_out, b_out):
    x = np.asarray(x, dtype=np.float32)
    w_qkv = np.asarray(w_qkv, dtype=np.float32)
    b_qkv = np.asarray(b_qkv, dtype=np.float32)
    w_out = np.asarray(w_out, dtype=np.float32)
    b_out = np.asarray(b_out, dtype=np.float32)

    if "nc" not in _CACHED:
        _CACHED["nc"] = _build()
    nc = _CACHED["nc"]

    # host-side layouts (shared across the 4 cores of each batch)
    xTs = [np.ascontiguousarray(
        x[b_].T.reshape(KC, P, T)).astype(BF_NP) for b_ in range(B)]
    bo = np.ascontiguousarray(b_out[None, :])

    def wslice(m, g):
        # [C, DL] -> [P, KC, DL]
        return np.ascontiguousarray(
            m.reshape(KC, P, DL).transpose(1, 0, 2)).astype(BF_NP)

    def bcol(v):
        # (DL,) -> [P, 2] with dim = mi*128 + p
        return np.ascontiguousarray(v.reshape(2, P).T)

    in_maps = []
    for c in range(8):
        b_, g = c // 4, c % 4
        sl = slice(g * DL, (g + 1) * DL)
        in_maps.append({
            "xT": xTs[b_],
            "wq": wslice(w_qkv[:, 0 * C:1 * C][:, sl], g),
            "wk": wslice(w_qkv[:, 1 * C:2 * C][:, sl], g),
            "wv": wslice(w_qkv[:, 2 * C:3 * C][:, sl], g),
            "bq": bcol(b_qkv[0 * C:1 * C][sl]),
            "bk": bcol(b_qkv[1 * C:2 * C][sl]),
            "bv": np.ascontiguousarray(b_qkv[2 * C:3 * C][sl][None, :]),
            "wo": np.ascontiguousarray(
                w_out[g * DL:(g + 1) * DL, :].reshape(2, P, C)
                .transpose(1, 0, 2)).astype(BF_NP),
            "bo": bo,
        })
    res = run_bass_kernel_spmd(nc, in_maps, list(range(8)))
    _CACHED["last_result"] = res
    out_full = np.empty((B, T, C), dtype=np.float32)
    for c in range(8):
        b_, g = c // 4, c % 4
        o = res.results[c]["out"]          # [NQ, P, C]
        for qc in range(NQ):
            r0 = qc * QB + g * P
            out_full[b_, r0:r0 + P, :] = o[qc]
    return out_full


# revision 14
# speedup vs baseline: 1.1536x; 1.0295x over previous
"""Distributed causal self-attention for 8 Trainium2 NeuronCores.

Problem: x[2,2048,1024] @ w_qkv[1024,3072] -> causal MHA (16 heads, d=64)
         -> @ w_out[1024,1024]. All fp32 in/out.

Sharding: core c (0..7) handles batch b=c//4 and head group g=c%4 (4 heads).
Each core projects qkv for its heads, runs flash attention (transposed-score
layout), then a ReduceScatter within each 4-core batch group combines the
partial output projections.  Core c owns output rows [b, qc*512+g*128, :].

v2 layout: all matmul operands in bf16 (fp32 PSUM accumulation, fp32
softmax math).  Projection is streamed per 512-token chunk so attention
for chunk qc overlaps the DMA + projection of chunk qc+1.  Score matmuls
for a head pair run row-packed (tile rows 0-63 / 64-127 concurrently);
pv matmuls are split per 64-key half and cross-packed over the pair.
"""

import sys

for _p in ("/opt/trn_rl_repo", "/root/.axon_site/_ro/trn_rl_repo"):
    if _p not in sys.path:
        sys.path.insert(0, _p)

import ml_dtypes
import numpy as np

import concourse.bass as bass  # noqa: F401
import concourse.mybir as mybir
import concourse.tile as tile
from concourse import bacc
from concourse.bass_utils import run_bass_kernel_spmd

P = 128
B, T, C = 2, 2048, 1024
H, D = 16, 64
HL = 4               # heads per core
DL = HL * D          # 256 local head dims
KC = C // P          # 8 contraction tiles over C
QB = 512             # query chunk
NQ = T // QB         # 4 query chunks
NT = T // P          # 16 token tiles
G = 4                # cores per batch group
SCALE = 1.0 / 8.0    # 1/sqrt(64)
NEG = -1.0e30
N_WARM = 10          # PE warm-up dummy matmuls

F32 = mybir.dt.float32
BF16 = mybir.dt.bfloat16
BF_NP = ml_dtypes.bfloat16

import os as _os
# conservative-mode bisection flags (1 = use baseline-style construct)
C_RECIP = _os.environ.get("K_C_RECIP", "0") == "1"   # ACT copy before recip
C_MASK = _os.environ.get("K_C_MASK", "0") == "1"     # per-head mask adds
C_EXP = _os.environ.get("K_C_EXP", "0") == "1"       # per-head 2D exp
C_PV = _os.environ.get("K_C_PV", "1") == "1"         # unpacked full-row pv
# (cross-packed pv — two row-tiled matmuls accumulating into the same
# PSUM tile — crashes the device; keep full-row pv.)
C_WARM = _os.environ.get("K_C_WARM", "0") == "1"     # no warmup dummies

_CACHED = {}


def _mask_data():
    # tril mask: 0 where key j <= query i, NEG above the diagonal
    j = np.arange(P)[:, None]
    i = np.arange(P)[None, :]
    return np.where(j <= i, 0.0, NEG).astype(np.float32)


def _build():
    nc = bacc.Bacc("TRN2", target_bir_lowering=False, debug=False,
                   num_devices=8)

    xT = nc.dram_tensor("xT", [KC, P, T], BF16, kind="ExternalInput")
    wq = nc.dram_tensor("wq", [P, KC, DL], BF16, kind="ExternalInput")
    wk = nc.dram_tensor("wk", [P, KC, DL], BF16, kind="ExternalInput")
    wv = nc.dram_tensor("wv", [P, KC, DL], BF16, kind="ExternalInput")
    bq = nc.dram_tensor("bq", [P, 2], F32, kind="ExternalInput")
    bk = nc.dram_tensor("bk", [P, 2], F32, kind="ExternalInput")
    bv = nc.dram_tensor("bv", [1, DL], F32, kind="ExternalInput")
    wo = nc.dram_tensor("wo", [P, 2, C], BF16, kind="ExternalInput")
    bo = nc.dram_tensor("bo", [1, C], F32, kind="ExternalInput")
    # per query-chunk ReduceScatter slices: rows qc*512 + g*128 .. +128
    out = nc.dram_tensor("out", [NQ, P, C], F32, kind="ExternalOutput")

    masks_dram = nc.inline_tensor(_mask_data(), name="cmasks")

    with tile.TileContext(nc) as tc:
        with (
            tc.tile_pool(name="const", bufs=1) as cp,
            tc.tile_pool(name="persist", bufs=1) as pp,
            tc.tile_pool(name="work", bufs=3) as wk_p,
            tc.tile_pool(name="xchunk", bufs=2) as xp,
            tc.tile_pool(name="dram", bufs=1, space="DRAM") as dp,
            tc.tile_pool(name="ps_a", bufs=2, space="PSUM") as ps_a,
            tc.tile_pool(name="ps_sT", bufs=2, space="PSUM") as ps_sT,
            tc.tile_pool(name="ps_pv", bufs=2, space="PSUM") as ps_pv,
        ):
            # ---- small constants ----
            masks = cp.tile([P, P], F32)
            nc.sync.dma_start(masks[:], masks_dram[:])
            bq_col = cp.tile([P, 2], F32)
            bk_col = cp.tile([P, 2], F32)
            nc.sync.dma_start(bq_col[:], bq[:])
            nc.sync.dma_start(bk_col[:], bk[:])
            bv_sb = cp.tile([1, DL], F32)
            bo_sb = cp.tile([1, C], F32)
            nc.sync.dma_start(bv_sb[:], bv[:])
            nc.sync.dma_start(bo_sb[:], bo[:])
            bv_bc = cp.tile([P, DL], F32)
            nc.gpsimd.partition_broadcast(bv_bc[:], bv_sb[:])
            bo_bc = cp.tile([P, C], F32)
            nc.gpsimd.partition_broadcast(bo_bc[:], bo_sb[:])
            spin = cp.tile([P, QB], BF16)
            nc.vector.memset(spin[:], 0.0)

            # ---- persistent activations (all bf16) ----
            qT_sb = pp.tile([P, 2, T], BF16)     # [d, t], d = mi*128+p
            kT_sb = pp.tile([P, 2, T], BF16)
            v_sb = pp.tile([P, NT, HL * (D + 1)], BF16)  # per head: 64 v + 1
            aoT_sb = pp.tile([P, 2, T], BF16)    # attention out^T (normalized)

            # ones columns of v_sb (softmax denominator accumulator)
            ones64 = cp.tile([P, NT * HL], F32)
            nc.vector.memset(ones64[:], 1.0)
            vones = v_sb.rearrange("p n (h e) -> p n h e", h=HL)[:, :, :, D:D + 1]
            nc.vector.tensor_copy(vones, ones64[:].rearrange(
                "p (n h) -> p n h", n=NT)[:, :, :, None])

            # ---- weights: per-kk DMA so the first matmuls start early ----
            wq_sb = pp.tile([P, KC, DL], BF16)
            wk_sb = pp.tile([P, KC, DL], BF16)
            wv_sb = pp.tile([P, KC, DL], BF16)
            for kk in range(KC):
                nc.sync.dma_start(wq_sb[:, kk, :], wq[:, kk, :])
                nc.sync.dma_start(wk_sb[:, kk, :], wk[:, kk, :])
                nc.sync.dma_start(wv_sb[:, kk, :], wv[:, kk, :])
            wo_sb = pp.tile([P, 2, C], BF16)
            nc.sync.dma_start(wo_sb[:], wo[:])

            # ---- PE warm-up: keep the HAM activity window busy during the
            # initial weight/x DMAs so real matmuls start at 2.4 GHz.
            if not C_WARM:
                for _ in range(N_WARM):
                    psd = ps_a.tile([P, QB], F32, name="psA", tag="psA")
                    nc.tensor.matmul(psd[:], spin[:, 0:P], spin[:],
                                     start=True, stop=True)

            def load_chunk(ni):
                xc = xp.tile([P, KC, QB], BF16, name="xc", tag="xc")
                for kk in range(KC):
                    nc.sync.dma_start(xc[:, kk, :],
                                      xT[kk, :, ni * QB:(ni + 1) * QB])
                return xc

            part_dram = dp.tile([T, C], BF16)
            rs_out = dp.tile([NQ, P, C], BF16)

            pending = []
            finishes = []

            def drain():
                if pending:
                    pending.pop(0)()

            def rs_trigger(qc):
                nc.gpsimd.collective_compute(
                    "ReduceScatter",
                    mybir.AluOpType.add,
                    replica_groups=[[0, 1, 2, 3], [4, 5, 6, 7]],
                    ins=[part_dram[qc * QB:(qc + 1) * QB, :]],
                    outs=[rs_out[qc]],
                )

            def rs_finish(qc):
                # post-RS: load the reduced slice, add the output bias,
                # store.  Deferred ~a full chunk after the trigger so the
                # DVE FIFO never blocks on the collective's latency.
                r_sb = wk_p.tile([P, C], BF16, name="r_sb",
                                 tag="r_sb", bufs=2)
                nc.sync.dma_start(r_sb[:], rs_out[qc])
                f_sb = wk_p.tile([P, C], F32, name="f_sb",
                                 tag="f_sb", bufs=2)
                nc.vector.tensor_add(f_sb[:], r_sb[:], bo_bc[:])
                nc.sync.dma_start(out[qc], f_sb[:])

            def outproj_jobs(qc):
                # 8 projection psum-groups + RS trigger for chunk qc;
                # emitted one at a time inside the NEXT chunk's stream as
                # PE gap filler.
                jobs = []

                def group(mi2, nj):
                    ps = ps_a.tile([P, QB], F32, name="psA", tag="psA")
                    for kk in range(2):
                        nc.tensor.matmul(
                            ps[:],
                            aoT_sb[:, kk, mi2 * P:(mi2 + 1) * P],
                            wo_sb[:, kk, nj * QB:(nj + 1) * QB],
                            start=(kk == 0), stop=(kk == 1))
                    o_sb = wk_p.tile([P, QB], BF16, name="o_sb",
                                     tag="o_sb", bufs=2)
                    nc.vector.tensor_copy(o_sb[:], ps[:])
                    nc.sync.dma_start(
                        part_dram[mi2 * P:(mi2 + 1) * P,
                                  nj * QB:(nj + 1) * QB],
                        o_sb[:])

                for mi2 in range(4 * qc, 4 * qc + 4):
                    for nj in range(2):
                        jobs.append(lambda mi2=mi2, nj=nj: group(mi2, nj))
                jobs.append(lambda: rs_trigger(qc))
                return jobs

            xcs = [load_chunk(0)]
            for ni in range(NQ):
                if ni + 1 < NQ:
                    xcs.append(load_chunk(ni + 1))
                xc = xcs[ni]

                # ---- q/k projection for this token chunk ----
                for w_sb, b_col, dst in ((wq_sb, bq_col, qT_sb),
                                         (wk_sb, bk_col, kT_sb)):
                    for mi in range(2):
                        ps = ps_a.tile([P, QB], F32, name="psA", tag="psA")
                        for kk in range(KC):
                            nc.tensor.matmul(
                                ps[:],
                                w_sb[:, kk, mi * P:(mi + 1) * P],
                                xc[:, kk, :],
                                start=(kk == 0), stop=(kk == KC - 1))
                        nc.vector.tensor_scalar_add(
                            dst[:, mi, ni * QB:(ni + 1) * QB], ps[:],
                            b_col[:, mi:mi + 1])
                        drain()
                # ---- v projection (tokens stationary) ----
                for tj in range(4):
                    ti = 4 * ni + tj
                    ps = ps_a.tile([P, QB], F32, name="psA", tag="psA")
                    for kk in range(KC):
                        nc.tensor.matmul(
                            ps[:, 0:DL],
                            xc[:, kk, tj * P:(tj + 1) * P],
                            wv_sb[:, kk, :],
                            start=(kk == 0), stop=(kk == KC - 1))
                    nc.vector.tensor_add(
                        v_sb.rearrange("p n (h e) -> p n h e", h=HL)
                        [:, ti, :, 0:D],
                        ps[:, 0:DL].rearrange("p (h e) -> p h e", e=D),
                        bv_bc[:].rearrange("p (h e) -> p h e", e=D))
                    drain()

                # ---- attention for query chunk qc = ni ----
                qc = ni
                nkb = 4 * qc + 4
                for pair in range(2):
                    mi = pair
                    he, ho = 2 * pair, 2 * pair + 1
                    pv = [ps_pv.tile([P, QB], F32, name="pv", tag="pv")
                          for _ in range(2)]
                    for kb in range(nkb):
                        diag = kb >= 4 * qc
                        q0 = P * (kb - 4 * qc) if diag else 0
                        sT = ps_sT.tile([P, 2, QB], F32, name="sT", tag="sT")
                        for e in range(2):
                            po = 64 * e
                            nc.tensor.matmul(
                                sT[:, e, q0:QB],
                                kT_sb[po:po + D, mi, kb * P:(kb + 1) * P],
                                qT_sb[po:po + D, mi,
                                      qc * QB + q0:(qc + 1) * QB],
                                start=True, stop=True)
                        if diag:
                            if C_MASK:
                                for e in range(2):
                                    nc.vector.tensor_add(
                                        sT[:, e, q0:q0 + P],
                                        sT[:, e, q0:q0 + P], masks[:])
                            else:
                                nc.vector.tensor_add(
                                    sT[:, :, q0:q0 + P], sT[:, :, q0:q0 + P],
                                    masks[:][:, None, :].broadcast_to(
                                        (P, 2, P)))
                        pT = wk_p.tile([P, 2, QB], BF16, name="pT", tag="pT")
                        if C_EXP:
                            for e in range(2):
                                nc.scalar.activation(
                                    pT[:, e, q0:QB], sT[:, e, q0:QB],
                                    mybir.ActivationFunctionType.Exp,
                                    scale=SCALE)
                        else:
                            nc.scalar.activation(
                                pT[:, :, q0:QB], sT[:, :, q0:QB],
                                mybir.ActivationFunctionType.Exp, scale=SCALE)
                        first, last = (kb == 0), (kb == nkb - 1)
                        if C_PV:
                            for e, h in ((0, he), (1, ho)):
                                nc.tensor.matmul(
                                    pv[e][0:D + 1, q0:QB],
                                    v_sb[:, kb, h * (D + 1):(h + 1) * (D + 1)],
                                    pT[:, e, q0:QB],
                                    start=first, stop=last)
                        else:
                            # cross-packed pv: slot 0 = (he, keys 0:64) ||
                            # (ho, keys 64:128); slot 1 swaps the key halves.
                            for slot in range(2):
                                ke, ko = 64 * slot, 64 - 64 * slot
                                nc.tensor.matmul(
                                    pv[0][0:D + 1, q0:QB],
                                    v_sb[ke:ke + 64, kb,
                                         he * (D + 1):(he + 1) * (D + 1)],
                                    pT[ke:ke + 64, 0, q0:QB],
                                    start=(first and slot == 0),
                                    stop=(last and slot == 1))
                                nc.tensor.matmul(
                                    pv[1][0:D + 1, q0:QB],
                                    v_sb[ko:ko + 64, kb,
                                         ho * (D + 1):(ho + 1) * (D + 1)],
                                    pT[ko:ko + 64, 1, q0:QB],
                                    start=(first and slot == 0),
                                    stop=(last and slot == 1))
                        drain()
                    # ---- normalize the pair: 1/l broadcast, ao = pv * r ----
                    rr = wk_p.tile([1, 2, QB], F32, name="rr", tag="rr",
                                   bufs=2)
                    # reciprocal_approx_fast must NOT read PSUM directly
                    # (silently wrong on HW); stage the row through SBUF.
                    lrow = wk_p.tile([1, 2, QB], F32, name="lrow",
                                     tag="lrow", bufs=2)
                    for e in range(2):
                        nc.vector.tensor_copy(lrow[0:1, e, :],
                                              pv[e][D:D + 1, :])
                        nc.vector.reciprocal_approx_fast(
                            out=rr[0:1, e, :], in_=lrow[0:1, e, :])
                    rbc = wk_p.tile([P, 2, QB], F32, name="rbc", tag="rbc",
                                    bufs=2)
                    nc.gpsimd.partition_broadcast(rbc[:], rr[:])
                    for e in range(2):
                        po = 64 * e
                        nc.vector.tensor_mul(
                            aoT_sb[po:po + D, mi, qc * QB:(qc + 1) * QB],
                            pv[e][0:D, :], rbc[po:po + D, e, :])

                assert not pending
                pending = outproj_jobs(qc)
                # finish the RS triggered two chunks ago (its data has
                # had ~two chunks of compute to land)
                finishes.append(qc)
                if len(finishes) >= 3:
                    rs_finish(finishes.pop(0))

            # drain the last chunk's jobs + outstanding RS finishes
            while pending:
                drain()
            for qc in finishes:
                rs_finish(qc)

    nc.compile()
    return nc


def kernel(x, w_qkv, b_qkv, w_out, b_out):
    x = np.asarray(x, dtype=np.float32)
    w_qkv = np.asarray(w_qkv, dtype=np.float32)
    b_qkv = np.asarray(b_qkv, dtype=np.float32)
    w_out = np.asarray(w_out, dtype=np.float32)
    b_out = np.asarray(b_out, dtype=np.float32)

    if "nc" not in _CACHED:
        _CACHED["nc"] = _build()
    nc = _CACHED["nc"]

    # host-side layouts (shared across the 4 cores of each batch)
    xTs = [np.ascontiguousarray(
        x[b_].T.reshape(KC, P, T)).astype(BF_NP) for b_ in range(B)]
    bo = np.ascontiguousarray(b_out[None, :])

    def wslice(m, g):
        # [C, DL] -> [P, KC, DL]
        return np.ascontiguousarray(
            m.reshape(KC, P, DL).transpose(1, 0, 2)).astype(BF_NP)

    def bcol(v):
        # (DL,) -> [P, 2] with dim = mi*128 + p
        return np.ascontiguousarray(v.reshape(2, P).T)

    in_maps = []
    for c in range(8):
        b_, g = c // 4, c % 4
        sl = slice(g * DL, (g + 1) * DL)
        in_maps.append({
            "xT": xTs[b_],
            "wq": wslice(w_qkv[:, 0 * C:1 * C][:, sl], g),
            "wk": wslice(w_qkv[:, 1 * C:2 * C][:, sl], g),
            "wv": wslice(w_qkv[:, 2 * C:3 * C][:, sl], g),
            "bq": bcol(b_qkv[0 * C:1 * C][sl]),
            "bk": bcol(b_qkv[1 * C:2 * C][sl]),
            "bv": np.ascontiguousarray(b_qkv[2 * C:3 * C][sl][None, :]),
            "wo": np.ascontiguousarray(
                w_out[g * DL:(g + 1) * DL, :].reshape(2, P, C)
                .transpose(1, 0, 2)).astype(BF_NP),
            "bo": bo,
        })
    res = run_bass_kernel_spmd(nc, in_maps, list(range(8)))
    _CACHED["last_result"] = res
    out_full = np.empty((B, T, C), dtype=np.float32)
    for c in range(8):
        b_, g = c // 4, c % 4
        o = res.results[c]["out"]          # [NQ, P, C]
        for qc in range(NQ):
            r0 = qc * QB + g * P
            out_full[b_, r0:r0 + P, :] = o[qc]
    return out_full
